# revision 102
# baseline (speedup 1.0000x reference)
"""Trainium2 Bass kernel for multi-head causal self-attention.

Problem: B=4, T=1024, D=2048, H=16 heads, E=128 head_dim, fp32 I/O.
  q/k/v = per-head projections of x; scores = causal-masked softmax(q k^T / sqrt(E));
  y = probs @ v; out = concat-heads(y) @ Wo^T + bo.

Sharding: 8 cores = 4 batches x 2 head-halves. Core c handles batch c//2 and
heads (c%2)*8 .. (c%2)*8+7. Each core computes its heads' q/k/v projections,
attention, and a partial out-projection (y_span @ Wo[:, span]^T) -> [D, T]
partial transposed output (bf16). Host sums the two half partials per batch
and adds the folded bias.

Precision scheme: QKV projections and out-projection run as fp8e4m3 hi/lo
DoubleRow products. q/k are W_hi @ x_hi only (their rounding error sits at the
softmax score-noise floor); v keeps three terms (W_hi@x_hi + W_lo@x_hi +
W_hi@x_lo) since its error passes straight to the output. The out-projection
keeps three terms likewise. Attention itself (scores via k-dup DR fp8,
P@V + row-sum ones-matmuls in bf16) is unchanged from the 2-term scheme.

Schedule: QKV projection (PE-heavy, ~8.5us/head) and attention (ACT/DVE-heavy,
~4.8us PE/head) are software-pipelined per head: slot s emits head s's
projections interleaved with head s-1's attention steps, so exp/normalize
latency hides under projection matmuls and PE stays the single bottleneck.
PSUM: projection ring 2 banks + scores 2 + y 2 + r 2 = 8.

Bias folding (host side):
  - bk: softmax-invariant -> dropped.
  - bv: rows of probs sum to 1 -> folded into bo_total = bo + Wo @ concat(bv).
  - bq: applied on-device during q eviction (scaled).
"""

import numpy as np

B, T, D, H = 4, 1024, 2048, 16
E = D // H            # 128
P = 128
ND = D // P           # 16 d-tiles
NT = T // P           # 8 t-blocks / q-blocks / k-tiles
HL = H // 2           # 8 heads per core
SCALE = 1.0 / np.sqrt(E)

EX, EW = 4, 12        # fp8 pre-scales for x and W (QKV)
EY, EWO = 5, 12       # fp8 pre-scales for y and Wo (out-proj)
EQ, EK = 5, 5         # fp8 pre-scales for q and k (scores)
ASC = 2.0 ** (-(EX + EW))       # QKV PSUM descale
QSC8 = float(SCALE * ASC * 2.0 ** EQ)   # q eviction: (psum + bq') * QSC8
KSC8 = float(ASC * 2.0 ** EK)           # k eviction scale
SSC = 2.0 ** (-(EQ + EK))       # scores PSUM descale (exp input scale)
CSC = 2.0 ** (-(EY + EWO))      # out-proj PSUM descale
RV = 2.0 ** (-EY)               # ones value: recip(r*RV) = 2^EY / r
# fp8 row-sum path: ET8 = fp8(RC * ET16); RC corrects the measured mean
# rounding bias of fp8(exp) sums (calibrated offline on this distribution)
RC = 1.000973

_cache = {}


def _build():
    import concourse.bass as bass
    import concourse.mybir as mybir
    import concourse.tile as tile
    from concourse import bacc
    from concourse.bass import ts
    from concourse.masks import make_identity

    F32 = mybir.dt.float32
    BF16 = mybir.dt.bfloat16
    FP8 = mybir.dt.float8e4
    AF = mybir.ActivationFunctionType
    OP = mybir.AluOpType
    DR = mybir.MatmulPerfMode.DoubleRow

    nc = bacc.Bacc("TRN2", target_bir_lowering=False, debug=False)

    xhi_d = nc.dram_tensor("xhi", [ND, P, T], FP8, kind="ExternalInput").ap()
    xlo_d = nc.dram_tensor("xlo", [ND, P, T], FP8, kind="ExternalInput").ap()
    # proj-major hi weights (k, q, v) so each projection is one flat DMA
    whi_d = nc.dram_tensor("whi", [HL, 3, P, ND * P], FP8, kind="ExternalInput").ap()
    # W_lo only needed for v (q/k run W_hi-only)
    wlo_d = nc.dram_tensor("wlo", [HL, P, ND * P], FP8, kind="ExternalInput").ap()
    bqT_d = nc.dram_tensor("bqT", [P, HL], F32, kind="ExternalInput").ap()
    # hi+lo packed per o-block: one DMA per block in phase C
    wot_d = nc.dram_tensor("wot", [ND, 2, P, HL * P], FP8,
                           kind="ExternalInput").ap()
    outT_d = nc.dram_tensor("outT", [ND, P, T], BF16, kind="ExternalOutput").ap()

    with tile.TileContext(nc) as tc:
        with (
            tc.tile_pool(name="const", bufs=1) as const,
            tc.tile_pool(name="qkv", bufs=1) as qkv,
            tc.tile_pool(name="etp", bufs=2) as etp,
            tc.tile_pool(name="rbp", bufs=2) as rbp,
            tc.tile_pool(name="ytp", bufs=4) as ytp,
            tc.tile_pool(name="small", bufs=4) as small,
            tc.tile_pool(name="yTp", bufs=1) as yTp,
            tc.tile_pool(name="wop", bufs=8) as wop,
        ):
            ones_f = const.tile([P, P], F32)
            nc.vector.memset(ones_f[:], RV)
            ones16 = const.tile([P, P], BF16)
            nc.vector.tensor_copy(ones16[:], ones_f[:])
            # fp8 ones for the DR pair row-sums (each tile counted once)
            ones8 = const.tile([P, P], FP8)
            nc.vector.memset(ones8[:], RV)
            bqT_t = const.tile([P, HL], F32)

            q8 = qkv.tile([P, HL, 2, T], FP8, tag="q8")  # [e, head, hi/lo, t]
            k8 = qkv.tile([P, HL, T], FP8, tag="k8")     # [e, head, t]
            # v transposed per head/k-tile: v_h[p, hl, j, e] = vT[e, hl, j*128+p]
            v_h = qkv.tile([P, HL, NT, P], BF16, tag="vh")

            yhi = yTp.tile([P, HL, T], FP8, tag="yhi")  # y*2^EY hi
            ylo = yTp.tile([P, HL, T], FP8, tag="ylo")  # residual

            def wot_dma(ob, q=None):
                # one hi+lo transfer per o-block; rides SP after the fused
                # region's weight traffic dies down (tile_wait_until keeps
                # the scheduler from hoisting it into the DMA-bound startup)
                wot_t = wop.tile([P, 2, HL, P], FP8, tag="wo", bufs=6,
                                 name="wot_t")
                with tc.tile_wait_until(0.055 + 0.002 * ob):
                    (q or nc.sync).dma_start(
                        wot_t[:], wot_d[ob].rearrange(
                            "two p (i f) -> p two i f", i=HL)
                    )
                return wot_t[:, 0], wot_t[:, 1]

            # ------------- fused per-head projection + attention -------------
            with (
                tc.tile_pool(name="xTp", bufs=1) as xTp,
                tc.tile_pool(name="w3p", bufs=1) as w3p,
                tc.tile_pool(name="qtp", bufs=4) as qtp,
                tc.tile_pool(name="ps_a", bufs=2, space="PSUM") as ps_a,
                tc.tile_pool(name="ps_s", bufs=1, space="PSUM") as ps_s,
                tc.tile_pool(name="ps_ar", bufs=2, space="PSUM") as ps_ar,
            ):
                xhi_t = xTp.tile([P, ND, T], FP8, tag="xhi")
                xlo_t = xTp.tile([P, ND, T], FP8, tag="xlo")
                vT = xTp.tile([P, HL, T], BF16, tag="vT")

                def wp_dma(hl, proj):
                    # one flat [P, ND*P] transfer per (head, projection)
                    wt = w3p.tile([P, ND, P], FP8, tag=f"w{proj}", bufs=2,
                                  name="wt")
                    nc.sync.dma_start(
                        wt[:], whi_d[hl, proj].rearrange("p (t f) -> p t f", t=ND))
                    return wt

                def wvl_dma(hl):
                    wt = w3p.tile([P, ND, P], FP8, tag="wvl", bufs=2, name="wvl")
                    nc.sync.dma_start(
                        wt[:], wlo_d[hl].rearrange("p (t f) -> p t f", t=ND))
                    return wt

                def w_head_dma(hl):
                    # issue in consumption order: k, q, v, v_lo
                    wk = wp_dma(hl, 1)
                    wq = wp_dma(hl, 0)
                    wv = wp_dma(hl, 2)
                    wl = wvl_dma(hl)
                    return {0: wq, 1: wk, 2: wv, "vl": wl}

                def x_quad_dma(dst, src, q4, q=None):
                    # one [P, 4, T] transfer per 4-d-tile group
                    (q or nc.sync).dma_start(
                        dst[:, 4 * q4:4 * q4 + 4, :],
                        src[4 * q4:4 * q4 + 4].rearrange("four p t -> p four t"))

                def x_half_dma(dst, src, q4, c, q=None):
                    # [P, 4, 512] c-half transfer: halves the arrival
                    # granularity so the first sweeps start sooner
                    (q or nc.sync).dma_start(
                        dst[:, 4 * q4:4 * q4 + 4, ts(c, 512)],
                        src[4 * q4:4 * q4 + 4, :, ts(c, 512)]
                        .rearrange("four p t -> p four t"))

                # startup: head-0 weights lead (slot 0 sweeps all three
                # projections per arriving x quad), then the x_hi stream,
                # then head-1's k, then x_lo (slot 0's hl terms), then the
                # rest of head 1. First transfers spread across issue queues
                # so their fixed issue latencies overlap.
                wk0 = wp_dma(0, 1)
                wq0 = wp_dma(0, 0)
                x_half_dma(xhi_t, xhi_d, 0, 0, nc.scalar)
                wv0 = wp_dma(0, 2)
                nc.sync.dma_start(bqT_t[:], bqT_d)
                x_half_dma(xhi_t, xhi_d, 1, 0, nc.gpsimd)
                wl0 = wvl_dma(0)
                x_half_dma(xhi_t, xhi_d, 2, 0, nc.scalar)
                x_half_dma(xhi_t, xhi_d, 3, 0)
                for q4 in range(4):
                    x_half_dma(xhi_t, xhi_d, q4, 1,
                               nc.scalar if q4 % 2 == 0 else None)
                w0 = {0: wq0, 1: wk0, 2: wv0, "vl": wl0}
                # head-1 k/q ride ahead of x_lo (their chunks run inside the
                # DMA-bound window); v/v_lo follow x_lo (consumed later)
                wk1 = wp_dma(1, 1)
                wq1 = wp_dma(1, 0)
                for q4 in range(4):
                    x_half_dma(xlo_t, xlo_d, q4, 0,
                               nc.scalar if q4 % 2 == 0 else None)
                for q4 in range(4):
                    x_half_dma(xlo_t, xlo_d, q4, 1,
                               nc.scalar if q4 % 2 == 0 else None)
                w1 = {0: wq1, 1: wk1, 2: wp_dma(1, 2),
                      "vl": wvl_dma(1)}
                # preload the ACT Exp table off the critical path
                dummy = small.tile([P, 1], F32, tag="racc", name="dummy")
                nc.scalar.activation(dummy[:], bqT_t[:, 0:1], AF.Exp)

                # ---- projection chunk emitters (head a) ----
                def emit_term(wd, ps, proj, c, term, start, stop):
                    """8 DR matmuls of one (proj, chunk, hi/lo term)."""
                    wt_full = wd["vl"] if term == 1 else wd[proj]
                    xt_full = xlo_t if term == 2 else xhi_t
                    for g in range(8):
                        nc.tensor.matmul(
                            ps[:], wt_full[:, 2 * g:2 * g + 2, :],
                            xt_full[:, 2 * g:2 * g + 2, ts(c, 512)],
                            start=(start and g == 0),
                            stop=(stop and g == 7),
                            perf_mode=DR,
                        )

                def emit_qk_chunk(wd, hl, proj, c):
                    """q or k: single W_hi term; evicted by caller."""
                    ps = ps_a.tile([P, 512], F32, tag="a", name="ps")
                    emit_term(wd, ps, proj, c, 0, True, True)
                    return ps

                def open_v_chunk(wd, c):
                    ps = ps_a.tile([P, 512], F32, tag="a", name="psv")
                    emit_term(wd, ps, 2, c, 0, True, False)
                    return ps

                def evict_chunk(hl, proj, c, ps):
                    if proj == 0:
                        nc.vector.tensor_scalar(
                            q8[:, hl, 0, ts(c, 512)], ps[:],
                            bqT_t[:, hl:hl + 1], QSC8,
                            op0=OP.add, op1=OP.mult,
                        )
                        qtmp = qtp.tile([P, 512], F32, tag="qtmp", name="qtmp")
                        nc.vector.tensor_scalar(
                            qtmp[:], ps[:],
                            bqT_t[:, hl:hl + 1], QSC8,
                            op0=OP.add, op1=OP.mult,
                        )
                        nc.vector.tensor_tensor(
                            q8[:, hl, 1, ts(c, 512)], qtmp[:],
                            q8[:, hl, 0, ts(c, 512)], op=OP.subtract,
                        )
                    elif proj == 1:
                        nc.scalar.activation(
                            k8[:, hl, ts(c, 512)], ps[:],
                            AF.Copy, scale=KSC8,
                        )
                    else:
                        nc.vector.tensor_scalar(
                            vT[:, hl, ts(c, 512)], ps[:],
                            float(ASC), 0.0, op0=OP.mult, op1=OP.add,
                        )
                        nc.sync.dma_start_transpose(
                            v_h[:, hl, 4 * c:4 * c + 4, :],
                            vT[:, hl, ts(c, 512)],
                        )

                # ---- attention step makers (head b) ----
                def make_B(hl, r8=True):
                    ET = etp.tile([P, NT, T], BF16, tag="ET", name="ET")
                    # fp8 copy of the cols>=512 half of ET, for the DR
                    # row-sums of r1 (long rows only: fp8 noise ~eps/sqrt(n))
                    ET8 = (etp.tile([P, NT, 512], FP8, tag="ET8", name="ET8")
                           if r8 else None)
                    rb = rbp.tile([P, T], F32, tag="rb", name="rb")
                    y0 = ps_ar.tile([P, 512], F32, tag="y", name="y0")
                    r0 = ps_ar.tile([P, 512], F32, tag="r", name="r0")
                    y1 = ps_ar.tile([P, 512], F32, tag="y", name="y1")
                    r1 = ps_ar.tile([P, 512], F32, tag="r", name="r1")

                    def CP(j):
                        # only the region where tile j's DR pair-partner is
                        # also post-diagonal; never touches the Pool affine's
                        # output, so ACT never bubbles
                        lo = 512 if j < 4 else 768
                        if j >= 6:
                            return
                        nc.scalar.activation(ET8[:, j, lo - 512:512],
                                             ET[:, j, lo:T], AF.Copy,
                                             scale=float(RC))

                    # cp placement tuned so exps 4-7 aren't pushed past the
                    # single-scores-buffer deadline and each cp lands just
                    # before its r1 consumer
                    CPS_AT_ST = {1: 0, 2: 1, 3: 2, 4: 3, 6: 4, 7: 5}

                    def ST(j):
                        if r8 and j in CPS_AT_ST:
                            CP(CPS_AT_ST[j])
                        # stride-0 repeat of the single k copy: the DR pair
                        # contracts [k; k] against [q_hi; q_lo]
                        kblk = k8[:, hl, ts(j, P)].unsqueeze(1).broadcast_to(
                            [P, 2, P])
                        # two independent 1-bank score tiles with separate
                        # exps: halves the WAR-serialization granularity
                        # between consecutive score tiles
                        if j < 4:
                            sA = ps_s.tile([P, 512], F32, tag="sA", name="sA")
                            nc.tensor.matmul(sA[:, j * P:512], kblk,
                                             q8[:, hl, 0:2, j * P:512],
                                             start=True, stop=True,
                                             perf_mode=DR)
                            nc.scalar.activation(ET[:, j, j * P:512],
                                                 sA[:, j * P:512], AF.Exp,
                                                 scale=float(SSC))
                        sB = ps_s.tile([P, 512], F32, tag="sB", name="sB")
                        lo = max(j * P, 512)
                        nc.tensor.matmul(sB[:, lo - 512:512], kblk,
                                         q8[:, hl, 0:2, lo:T],
                                         start=True, stop=True,
                                         perf_mode=DR)
                        nc.scalar.activation(ET[:, j, lo:T],
                                             sB[:, lo - 512:512], AF.Exp,
                                             scale=float(SSC))
                        nc.gpsimd.affine_select(
                            out=ET[:, j, j * P:(j + 1) * P],
                            in_=ET[:, j, j * P:(j + 1) * P],
                            compare_op=mybir.AluOpType.is_ge, fill=0.0,
                            base=0, pattern=[[1, P]], channel_multiplier=-1,
                        )

                    def AVR(jq):
                        if jq <= 3:
                            lo = jq * P
                            st, sp = jq == 0, jq == 3
                            nc.tensor.matmul(y0[:, lo:512], v_h[:, hl, jq, :],
                                             ET[:, jq, lo:512], start=st, stop=sp,
                                             skip_group_check=True)
                            nc.tensor.matmul(r0[:, lo:512], ones16[:],
                                             ET[:, jq, lo:512], start=st, stop=sp,
                                             skip_group_check=True)
                        lo = max(jq * P, 512)
                        st, sp = jq == 0, jq == NT - 1
                        nc.tensor.matmul(y1[:, lo - 512:512], v_h[:, hl, jq, :],
                                         ET[:, jq, lo:T], start=st, stop=sp,
                                         skip_group_check=True)
                        if not r8:
                            nc.tensor.matmul(r1[:, lo - 512:512], ones16[:],
                                             ET[:, jq, lo:T], start=st, stop=sp,
                                             skip_group_check=True)
                            return
                        # r1 via fp8 DR on adjacent ET8 tile-pairs where both
                        # tiles are post-diagonal; diagonal tiles and tile 6's
                        # unpaired region ride bf16. Emitted at the latest AVR
                        # step whose CPs have landed.
                        if jq == 1:
                            nc.tensor.matmul(
                                r1[:, 0:512],
                                ones8[:].unsqueeze(1).broadcast_to([P, 2, P]),
                                ET8[:, 0:2, 0:512],
                                start=True, stop=False,
                                perf_mode=DR, skip_group_check=True)
                        elif jq == 3:
                            nc.tensor.matmul(
                                r1[:, 0:512],
                                ones8[:].unsqueeze(1).broadcast_to([P, 2, P]),
                                ET8[:, 2:4, 0:512],
                                start=False, stop=False,
                                perf_mode=DR, skip_group_check=True)
                        elif jq == 4:
                            # tile 4's diagonal plus its [640:768] region
                            # (pair (4,5) only covers cols >= 768)
                            nc.tensor.matmul(r1[:, 0:2 * P], ones16[:],
                                             ET[:, 4, 512:768],
                                             start=False, stop=False,
                                             skip_group_check=True)
                        elif jq == 5:
                            nc.tensor.matmul(r1[:, P:2 * P], ones16[:],
                                             ET[:, 5, 640:768],
                                             start=False, stop=False,
                                             skip_group_check=True)
                        elif jq == 6:
                            nc.tensor.matmul(
                                r1[:, 256:512],
                                ones8[:].unsqueeze(1).broadcast_to([P, 2, P]),
                                ET8[:, 4:6, 256:512],
                                start=False, stop=False,
                                perf_mode=DR, skip_group_check=True)
                            nc.tensor.matmul(r1[:, 2 * P:4 * P], ones16[:],
                                             ET[:, 6, 768:T],
                                             start=False, stop=False,
                                             skip_group_check=True)
                        elif jq == 7:
                            nc.tensor.matmul(r1[:, 3 * P:512], ones16[:],
                                             ET[:, 7, 896:T],
                                             start=False, stop=True,
                                             skip_group_check=True)

                    def norm3(dst_hi, dst_lo, y_ps, rb_ap, tag):
                        nc.vector.tensor_tensor(dst_hi, y_ps, rb_ap,
                                                op=OP.mult)
                        ytmp = ytp.tile([P, 512], F32, tag="yt",
                                        name="ytmp" + tag)
                        nc.vector.tensor_tensor(ytmp[:], y_ps, rb_ap,
                                                op=OP.mult)
                        nc.vector.tensor_tensor(dst_lo, ytmp[:], dst_hi,
                                                op=OP.subtract)

                    def tail0():
                        nc.vector.reciprocal(rb[:, 0:512], r0[:])
                        norm3(yhi[:, hl, 0:512], ylo[:, hl, 0:512],
                              y0[:], rb[:, 0:512], "0")

                    def tail1():
                        nc.vector.reciprocal(rb[:, 512:T], r1[:])
                        norm3(yhi[:, hl, 512:T], ylo[:, hl, 512:T],
                              y1[:], rb[:, 512:T], "1")

                    return ST, AVR, tail0, tail1

                # pre-issue the first three wot blocks early (they gate the
                # B(7)-trail weave and C start)
                wot_pre = [wot_dma(ob) for ob in range(2)]

                # ---- out-projection chunk helpers (shared with phase C) ----
                def emit_C_mms(wt_hi, wt_lo, c, o_ps):
                    for term in range(3):  # hh, lh, hl
                        wt = wt_lo if term == 1 else wt_hi
                        yt = ylo if term == 2 else yhi
                        for g in range(4):
                            nc.tensor.matmul(
                                o_ps[:],
                                wt[:, 2 * g:2 * g + 2, :],
                                yt[:, 2 * g:2 * g + 2, ts(c, 512)],
                                start=(term == 0 and g == 0),
                                stop=(term == 2 and g == 3),
                                perf_mode=DR,
                            )

                # ---- prologue: heads 0+1 under the DMA-bound window ----
                # Slot 0 sweeps quad-major with six open PSUM chunks (the
                # y/r banks are idle before attention starts); head 1's
                # x_hi-only chunks slip in before head 0's x_lo-fed terms so
                # PE never waits on the tail of the input stream.
                kc = [ps_a.tile([P, 512], F32, tag="a", name="kc")
                      for _ in range(2)]
                qc = [ps_ar.tile([P, 512], F32, tag="y", name="qc")
                      for _ in range(2)]
                vc = [ps_ar.tile([P, 512], F32, tag="r", name="vc")
                      for _ in range(2)]

                def mm0(ps, wt, g, c, xt, start, stop):
                    nc.tensor.matmul(
                        ps[:], wt[:, 2 * g:2 * g + 2, :],
                        xt[:, 2 * g:2 * g + 2, ts(c, 512)],
                        start=start, stop=stop, perf_mode=DR)

                for c in range(2):
                    for g in range(8):
                        mm0(kc[c], w0[1], g, c, xhi_t, g == 0, g == 7)
                        mm0(qc[c], w0[0], g, c, xhi_t, g == 0, g == 7)
                        mm0(vc[c], w0[2], g, c, xhi_t, g == 0, False)
                    evict_chunk(0, 1, c, kc[c])
                    evict_chunk(0, 0, c, qc[c])
                for c in range(2):
                    for g in range(8):
                        mm0(vc[c], w0["vl"], g, c, xhi_t, False, False)
                # B(0)'s first score tiles start as soon as head 0's q/k are
                # evicted, and head 0's x_lo-fed v terms run chunk-major so
                # each v half evicts + transposes at the earliest moment (the
                # transpose gates B(0)'s P@V steps). Head 1's first chunks
                # fill the x_lo arrival gaps.
                ST, AVR, tail0, tail1 = make_B(0)
                ST(0)
                for g in range(2):
                    mm0(vc[0], w0[2], g, 0, xlo_t, False, False)
                k10 = emit_qk_chunk(w1, 1, 1, 0)
                evict_chunk(1, 1, 0, k10)
                for g in range(2, 4):
                    mm0(vc[0], w0[2], g, 0, xlo_t, False, False)
                ST(1)
                q10 = emit_qk_chunk(w1, 1, 0, 0)
                evict_chunk(1, 0, 0, q10)
                for g in range(4, 8):
                    mm0(vc[0], w0[2], g, 0, xlo_t, False, g == 7)
                evict_chunk(0, 2, 0, vc[0])
                ST(2)
                for g in range(8):
                    mm0(vc[1], w0[2], g, 1, xlo_t, False, g == 7)
                evict_chunk(0, 2, 1, vc[1])
                w2 = w_head_dma(2)
                # head 1 remainder woven with head 0's attention
                k11 = emit_qk_chunk(w1, 1, 1, 1)
                ST(3)
                evict_chunk(1, 1, 1, k11)
                psv = open_v_chunk(w1, 0)
                AVR(0)
                emit_term(w1, psv, 2, 0, 1, False, False)
                AVR(1)
                ST(4)
                emit_term(w1, psv, 2, 0, 2, False, True)
                AVR(2)
                evict_chunk(1, 2, 0, psv)
                q11 = emit_qk_chunk(w1, 1, 0, 1)
                ST(5)
                evict_chunk(1, 0, 1, q11)
                AVR(3)
                tail0()
                psv = open_v_chunk(w1, 1)
                ST(6)
                AVR(4)
                emit_term(w1, psv, 2, 1, 1, False, False)
                ST(7)
                AVR(5)
                emit_term(w1, psv, 2, 1, 2, False, True)
                evict_chunk(1, 2, 1, psv)
                AVR(6)
                AVR(7)
                tail1()
                Bst = make_B(1)
                wd = w2

                # ---- the steady-state slot loop (heads 2..7) ----
                for a in range(2, HL):
                    if a + 1 < HL:
                        wn = w_head_dma(a + 1)  # prefetch next head (SP)
                    # weave head a's projections with head a-1's
                    # attention. k-evictions go on ACT ahead of the exps
                    # (the v chunk reuses k's PSUM bank); STs spaced by
                    # A-matmuls so the single scores buffer never gates.
                    ST, AVR, tail0, tail1 = Bst
                    ps = emit_qk_chunk(wd, a, 1, 0)  # k c0
                    evict_chunk(a, 1, 0, ps)
                    ps = emit_qk_chunk(wd, a, 0, 0)  # q c0
                    ST(0)
                    evict_chunk(a, 0, 0, ps)
                    psv = open_v_chunk(wd, 0)        # v c0 hh
                    ST(1)
                    AVR(0)
                    emit_term(wd, psv, 2, 0, 1, False, False)
                    ST(2)
                    AVR(1)
                    emit_term(wd, psv, 2, 0, 2, False, True)
                    ST(3)
                    evict_chunk(a, 2, 0, psv)
                    AVR(2)
                    ps = emit_qk_chunk(wd, a, 1, 1)  # k c1
                    ST(4)
                    evict_chunk(a, 1, 1, ps)
                    AVR(3)
                    tail0()
                    ps = emit_qk_chunk(wd, a, 0, 1)  # q c1
                    ST(5)
                    evict_chunk(a, 0, 1, ps)
                    AVR(4)
                    psv = open_v_chunk(wd, 1)        # v c1 hh
                    ST(6)
                    AVR(5)
                    emit_term(wd, psv, 2, 1, 1, False, False)
                    ST(7)
                    AVR(6)
                    emit_term(wd, psv, 2, 1, 2, False, True)
                    evict_chunk(a, 2, 1, psv)
                    AVR(7)
                    tail1()
                    # last head's attention runs un-fused (exp-chain bound):
                    # keep its row-sums on the cheap-latency bf16 path there
                    Bst = make_B(a, r8=(a < HL - 1))
                    if a + 1 < HL:
                        wd = wn

                # trailing attention for the last head; once its first-chunk
                # normalize lands, weave in the first two out-proj chunks to
                # hide the exp chain
                ST, AVR, tail0, tail1 = Bst
                ST(0)
                ST(1)
                AVR(0)
                ST(2)
                AVR(1)
                ST(3)
                AVR(2)
                ST(4)
                AVR(3)
                tail0()
                ST(5)
                AVR(4)
                o00 = ps_a.tile([P, 512], F32, tag="a", name="o00")
                emit_C_mms(wot_pre[0][0], wot_pre[0][1], 0, o00)
                ST(6)
                AVR(5)
                o10 = ps_a.tile([P, 512], F32, tag="a", name="o10")
                emit_C_mms(wot_pre[1][0], wot_pre[1][1], 0, o10)
                ST(7)
                AVR(6)
                AVR(7)
                tail1()
                for ob, o_ps in ((0, o00), (1, o10)):
                    out_sb = ytp.tile([P, 512], BF16, tag="osb0", name="osb0")
                    nc.scalar.activation(out_sb[:], o_ps[:], AF.Copy,
                                         scale=float(CSC))
                    nc.sync.dma_start(outT_d[ob, :, 0:512], out_sb[:])

            # -------- Phase C: partial out-projection (fp8 DR) --------
            with (
                tc.tile_pool(name="osb", bufs=4) as osb,
                tc.tile_pool(name="ps_o", bufs=4, space="PSUM") as ps_o,
            ):
                # (0,0)/(1,0) were woven into the B(7) trail; (0,1)/(1,1)
                # run last since their wot is prefetched (no DMA dependency
                # near the tail) and (0,1) must wait for the trail's second
                # normalize anyway
                chunk_order = [(2, 0), (2, 1)]
                chunk_order += [(ob, c) for ob in range(3, ND) for c in range(2)]
                chunk_order += [(0, 1), (1, 1)]
                wot_t = dict(enumerate(wot_pre))
                for ob, c in chunk_order:
                    if ob not in wot_t:
                        wot_t[ob] = wot_dma(ob)
                    wt_hi, wt_lo = wot_t[ob]
                    if True:
                        o_ps = ps_o.tile([P, 512], F32, tag="o")
                        for term in range(3):  # hh, lh, hl
                            wt = wt_lo if term == 1 else wt_hi
                            yt = ylo if term == 2 else yhi
                            for g in range(4):
                                nc.tensor.matmul(
                                    o_ps[:],
                                    wt[:, 2 * g:2 * g + 2, :],
                                    yt[:, 2 * g:2 * g + 2, ts(c, 512)],
                                    start=(term == 0 and g == 0),
                                    stop=(term == 2 and g == 3),
                                    perf_mode=DR,
                                )
                        last = (ob, c) == chunk_order[-1]
                        # the final chunk evicts in halves so its DMA-latency
                        # chain starts one copy earlier
                        for h in range(2 if last else 1):
                            hs = slice(h * 256, 256 + h * 256) if last \
                                else slice(0, 512)
                            out_sb = osb.tile([P, 512], BF16, tag="osb")
                            nc.scalar.activation(out_sb[:, hs], o_ps[:, hs],
                                                 AF.Copy, scale=float(CSC))
                            nc.sync.dma_start(
                                outT_d[ob, :, 512 * c + hs.start:
                                       512 * c + hs.stop],
                                out_sb[:, hs])

    nc.compile()
    return nc


def _get_compiled():
    if "nc" not in _cache:
        _cache["nc"] = _build()
    return _cache["nc"]


def _hilo(a, e):
    import ml_dtypes
    F8 = ml_dtypes.float8_e4m3
    s = np.asarray(a, np.float32) * np.float32(2.0 ** e)
    hi = s.astype(F8)
    lo = (s - hi.astype(np.float32)).astype(F8)
    return hi, lo


def _host_prep(x, Wq, bq, Wk, Wv, Wo):
    """Build per-core input maps."""
    in_maps = []
    # xT per batch: [D, T] -> [ND, P, T] fp8 hi/lo at scale 2^EX
    xTs = []
    for b in range(B):
        xh, xl = _hilo(np.ascontiguousarray(x[b].T).reshape(ND, P, T), EX)
        xTs.append((xh, xl))
    halves = []
    for half in range(2):
        hs = slice(half * HL, (half + 1) * HL)
        # WqT/WkT/WvT per head: [D, E] -> [ND, P, E]; pack so each 4-d-tile
        # group is one contiguous [P, 4*3P] DMA: [HL, 4, P, 4*3*P].
        # The lo stream only carries v (q/k are W_hi-only): [HL, 4, P, 4*P].
        w3s = np.empty((HL, ND, P, 3 * P), dtype=np.float32)
        for hl, h in enumerate(range(half * HL, (half + 1) * HL)):
            w3s[hl, :, :, 0:P] = Wq[h].T.reshape(ND, P, P)
            w3s[hl, :, :, P:2 * P] = Wk[h].T.reshape(ND, P, P)
            w3s[hl, :, :, 2 * P:3 * P] = Wv[h].T.reshape(ND, P, P)
        whi_s, wlo_s = _hilo(w3s, EW)
        # proj-major flat packs: [HL, 3, P, ND*P] hi, [HL, P, ND*P] v-lo
        whi = np.ascontiguousarray(
            np.asarray(whi_s).reshape(HL, ND, P, 3, P)
            .transpose(0, 3, 2, 1, 4)
        ).reshape(HL, 3, P, ND * P)
        wlo = np.ascontiguousarray(
            np.asarray(wlo_s)[:, :, :, 2 * P:3 * P].transpose(0, 2, 1, 3)
        ).reshape(HL, P, ND * P)
        bqT = np.ascontiguousarray(bq[hs].T) * np.float32(2.0 ** (EX + EW))
        # WoT span blocks: WoT = Wo.T [i, o]; rows i in this half's span
        WoT_span = Wo.T[half * 1024:(half + 1) * 1024]  # [1024, D]
        # pack to [ND(o-block), P, HL*P] so each o-block is one contiguous DMA
        wot = np.ascontiguousarray(
            WoT_span.reshape(HL, P, ND, P).transpose(2, 1, 0, 3)
        ).reshape(ND, P, HL * P)
        wothi, wotlo = _hilo(wot, EWO)
        wotp = np.ascontiguousarray(
            np.stack([np.asarray(wothi), np.asarray(wotlo)], axis=1))
        halves.append({"whi": whi, "wlo": wlo, "bqT": bqT, "wot": wotp})
    for c in range(8):
        b, half = c // 2, c % 2
        hv = halves[half]
        in_maps.append({"xhi": xTs[b][0], "xlo": xTs[b][1], "whi": hv["whi"],
                        "wlo": hv["wlo"], "bqT": hv["bqT"],
                        "wot": hv["wot"]})
    return in_maps


def _numpy_fallback(x, attention_mask, Wq, bq, Wk, bk, Wv, bv, Wo, bo):
    out = np.empty((B, T, D), dtype=np.float32)
    neg = np.float32(np.finfo(np.float32).min)
    for b in range(B):
        xb = x[b]
        q = np.einsum("td,hed->hte", xb, Wq) + bq[:, None, :]
        k = np.einsum("td,hed->hte", xb, Wk) + bk[:, None, :]
        v = np.einsum("td,hed->hte", xb, Wv) + bv[:, None, :]
        s = np.einsum("hqe,hke->hqk", q, k).astype(np.float32) * np.float32(SCALE)
        causal = np.arange(T)[None, :] > np.arange(T)[:, None]
        s = np.where(causal[None], neg, s)
        keep = attention_mask[b].astype(bool)
        s = np.where(keep[None, None, :], s, neg)
        s = s - s.max(-1, keepdims=True)
        p = np.exp(s)
        p = p / p.sum(-1, keepdims=True)
        y = np.einsum("hqk,hke->hqe", p, v)
        y = np.transpose(y, (1, 0, 2)).reshape(T, D)
        out[b] = y @ Wo.T + bo
    return out


def kernel(x, attention_mask, Wq, bq, Wk, bk, Wv, bv, Wo, bo):
    x = np.asarray(x, dtype=np.float32)
    attention_mask = np.asarray(attention_mask)
    Wq, bq = np.asarray(Wq, np.float32), np.asarray(bq, np.float32)
    Wk, bk = np.asarray(Wk, np.float32), np.asarray(bk, np.float32)
    Wv, bv = np.asarray(Wv, np.float32), np.asarray(bv, np.float32)
    Wo, bo = np.asarray(Wo, np.float32), np.asarray(bo, np.float32)

    if not np.all(attention_mask == 1):
        return _numpy_fallback(x, attention_mask, Wq, bq, Wk, bk, Wv, bv, Wo, bo)

    from concourse.bass_utils import run_bass_kernel_spmd

    nc = _get_compiled()
    in_maps = _host_prep(x, Wq, bq, Wk, Wv, Wo)
    res = run_bass_kernel_spmd(nc, in_maps, core_ids=list(range(8)))

    # bv folds through softmax (rows sum to 1); bk is softmax-invariant
    bo_total = (bo + Wo @ bv.reshape(D)).astype(np.float32)

    out = np.zeros((B, T, D), dtype=np.float32)
    for c in range(8):
        partial = np.asarray(res.results[c]["outT"]).astype(np.float32)
        out[c // 2] += partial.reshape(D, T).T
    out += bo_total
    return out


# revision 104
# speedup vs baseline: 1.0013x; 1.0013x over previous
"""Trainium2 Bass kernel for multi-head causal self-attention.

Problem: B=4, T=1024, D=2048, H=16 heads, E=128 head_dim, fp32 I/O.
  q/k/v = per-head projections of x; scores = causal-masked softmax(q k^T / sqrt(E));
  y = probs @ v; out = concat-heads(y) @ Wo^T + bo.

Sharding: 8 cores = 4 batches x 2 head-halves. Core c handles batch c//2 and
heads (c%2)*8 .. (c%2)*8+7. Each core computes its heads' q/k/v projections,
attention, and a partial out-projection (y_span @ Wo[:, span]^T) -> [D, T]
partial transposed output (bf16). Host sums the two half partials per batch
and adds the folded bias.

Precision scheme: QKV projections and out-projection run as fp8e4m3 hi/lo
DoubleRow products. q/k are W_hi @ x_hi only (their rounding error sits at the
softmax score-noise floor); v keeps three terms (W_hi@x_hi + W_lo@x_hi +
W_hi@x_lo) since its error passes straight to the output. The out-projection
keeps three terms likewise. Attention itself (scores via k-dup DR fp8,
P@V + row-sum ones-matmuls in bf16) is unchanged from the 2-term scheme.

Schedule: QKV projection (PE-heavy, ~8.5us/head) and attention (ACT/DVE-heavy,
~4.8us PE/head) are software-pipelined per head: slot s emits head s's
projections interleaved with head s-1's attention steps, so exp/normalize
latency hides under projection matmuls and PE stays the single bottleneck.
PSUM: projection ring 2 banks + scores 2 + y 2 + r 2 = 8.

Bias folding (host side):
  - bk: softmax-invariant -> dropped.
  - bv: rows of probs sum to 1 -> folded into bo_total = bo + Wo @ concat(bv).
  - bq: applied on-device during q eviction (scaled).
"""

import numpy as np

B, T, D, H = 4, 1024, 2048, 16
E = D // H            # 128
P = 128
ND = D // P           # 16 d-tiles
NT = T // P           # 8 t-blocks / q-blocks / k-tiles
HL = H // 2           # 8 heads per core
SCALE = 1.0 / np.sqrt(E)

EX, EW = 4, 12        # fp8 pre-scales for x and W (QKV)
EY, EWO = 5, 12       # fp8 pre-scales for y and Wo (out-proj)
EQ, EK = 5, 5         # fp8 pre-scales for q and k (scores)
ASC = 2.0 ** (-(EX + EW))       # QKV PSUM descale
QSC8 = float(SCALE * ASC * 2.0 ** EQ)   # q eviction: (psum + bq') * QSC8
KSC8 = float(ASC * 2.0 ** EK)           # k eviction scale
SSC = 2.0 ** (-(EQ + EK))       # scores PSUM descale (exp input scale)
CSC = 2.0 ** (-(EY + EWO))      # out-proj PSUM descale
RV = 2.0 ** (-EY)               # ones value: recip(r*RV) = 2^EY / r
# fp8 row-sum path: ET8 = fp8(RC * ET16); RC corrects the measured mean
# rounding bias of fp8(exp) sums (calibrated offline on this distribution)
RC = 1.000973

_cache = {}


def _build():
    import concourse.bass as bass
    import concourse.mybir as mybir
    import concourse.tile as tile
    from concourse import bacc
    from concourse.bass import ts
    from concourse.masks import make_identity

    F32 = mybir.dt.float32
    BF16 = mybir.dt.bfloat16
    FP8 = mybir.dt.float8e4
    AF = mybir.ActivationFunctionType
    OP = mybir.AluOpType
    DR = mybir.MatmulPerfMode.DoubleRow

    nc = bacc.Bacc("TRN2", target_bir_lowering=False, debug=False)

    xhi_d = nc.dram_tensor("xhi", [ND, P, T], FP8, kind="ExternalInput").ap()
    xlo_d = nc.dram_tensor("xlo", [ND, P, T], FP8, kind="ExternalInput").ap()
    # proj-major hi weights (k, q, v) so each projection is one flat DMA
    whi_d = nc.dram_tensor("whi", [HL, 3, P, ND * P], FP8, kind="ExternalInput").ap()
    # W_lo only needed for v (q/k run W_hi-only)
    wlo_d = nc.dram_tensor("wlo", [HL, P, ND * P], FP8, kind="ExternalInput").ap()
    bqT_d = nc.dram_tensor("bqT", [P, HL], F32, kind="ExternalInput").ap()
    # hi+lo packed per o-block: one DMA per block in phase C
    wot_d = nc.dram_tensor("wot", [ND, 2, P, HL * P], FP8,
                           kind="ExternalInput").ap()
    outT_d = nc.dram_tensor("outT", [ND, P, T], BF16, kind="ExternalOutput").ap()

    with tile.TileContext(nc) as tc:
        with (
            tc.tile_pool(name="const", bufs=1) as const,
            tc.tile_pool(name="qkv", bufs=1) as qkv,
            tc.tile_pool(name="etp", bufs=2) as etp,
            tc.tile_pool(name="rbp", bufs=2) as rbp,
            tc.tile_pool(name="ytp", bufs=4) as ytp,
            tc.tile_pool(name="small", bufs=4) as small,
            tc.tile_pool(name="yTp", bufs=1) as yTp,
            tc.tile_pool(name="wop", bufs=8) as wop,
        ):
            ones_f = const.tile([P, P], F32)
            nc.vector.memset(ones_f[:], RV)
            ones16 = const.tile([P, P], BF16)
            nc.vector.tensor_copy(ones16[:], ones_f[:])
            # fp8 ones for the DR pair row-sums (each tile counted once)
            ones8 = const.tile([P, P], FP8)
            nc.vector.memset(ones8[:], RV)
            bqT_t = const.tile([P, HL], F32)

            q8 = qkv.tile([P, HL, 2, T], FP8, tag="q8")  # [e, head, hi/lo, t]
            k8 = qkv.tile([P, HL, T], FP8, tag="k8")     # [e, head, t]
            # v transposed per head/k-tile: v_h[p, hl, j, e] = vT[e, hl, j*128+p]
            v_h = qkv.tile([P, HL, NT, P], BF16, tag="vh")

            yhi = yTp.tile([P, HL, T], FP8, tag="yhi")  # y*2^EY hi
            ylo = yTp.tile([P, HL, T], FP8, tag="ylo")  # residual

            def wot_dma(ob, q=None):
                # one hi+lo transfer per o-block; rides SP after the fused
                # region's weight traffic dies down (tile_wait_until keeps
                # the scheduler from hoisting it into the DMA-bound startup)
                wot_t = wop.tile([P, 2, HL, P], FP8, tag="wo", bufs=6,
                                 name="wot_t")
                with tc.tile_wait_until(0.055 + 0.002 * ob):
                    (q or nc.sync).dma_start(
                        wot_t[:], wot_d[ob].rearrange(
                            "two p (i f) -> p two i f", i=HL)
                    )
                return wot_t[:, 0], wot_t[:, 1]

            # ------------- fused per-head projection + attention -------------
            with (
                tc.tile_pool(name="xTp", bufs=1) as xTp,
                tc.tile_pool(name="w3p", bufs=1) as w3p,
                tc.tile_pool(name="qtp", bufs=4) as qtp,
                tc.tile_pool(name="ps_a", bufs=2, space="PSUM") as ps_a,
                tc.tile_pool(name="ps_s", bufs=1, space="PSUM") as ps_s,
                tc.tile_pool(name="ps_ar", bufs=2, space="PSUM") as ps_ar,
            ):
                xhi_t = xTp.tile([P, ND, T], FP8, tag="xhi")
                xlo_t = xTp.tile([P, ND, T], FP8, tag="xlo")
                vT = xTp.tile([P, HL, T], BF16, tag="vT")

                def wp_dma(hl, proj):
                    # one flat [P, ND*P] transfer per (head, projection)
                    wt = w3p.tile([P, ND, P], FP8, tag=f"w{proj}", bufs=2,
                                  name="wt")
                    nc.sync.dma_start(
                        wt[:], whi_d[hl, proj].rearrange("p (t f) -> p t f", t=ND))
                    return wt

                def wvl_dma(hl):
                    wt = w3p.tile([P, ND, P], FP8, tag="wvl", bufs=2, name="wvl")
                    nc.sync.dma_start(
                        wt[:], wlo_d[hl].rearrange("p (t f) -> p t f", t=ND))
                    return wt

                def w_head_dma(hl):
                    # issue in consumption order: k, q, v, v_lo
                    wk = wp_dma(hl, 1)
                    wq = wp_dma(hl, 0)
                    wv = wp_dma(hl, 2)
                    wl = wvl_dma(hl)
                    return {0: wq, 1: wk, 2: wv, "vl": wl}

                def x_quad_dma(dst, src, q4, q=None):
                    # one [P, 4, T] transfer per 4-d-tile group
                    (q or nc.sync).dma_start(
                        dst[:, 4 * q4:4 * q4 + 4, :],
                        src[4 * q4:4 * q4 + 4].rearrange("four p t -> p four t"))

                def x_half_dma(dst, src, q4, c, q=None):
                    # [P, 4, 512] c-half transfer: halves the arrival
                    # granularity so the first sweeps start sooner
                    (q or nc.sync).dma_start(
                        dst[:, 4 * q4:4 * q4 + 4, ts(c, 512)],
                        src[4 * q4:4 * q4 + 4, :, ts(c, 512)]
                        .rearrange("four p t -> p four t"))

                # startup: head-0 weights lead (slot 0 sweeps all three
                # projections per arriving x quad), then the x_hi stream,
                # then head-1's k, then x_lo (slot 0's hl terms), then the
                # rest of head 1. First transfers spread across issue queues
                # so their fixed issue latencies overlap.
                wk0 = wp_dma(0, 1)
                wq0 = wp_dma(0, 0)
                x_half_dma(xhi_t, xhi_d, 0, 0, nc.scalar)
                wv0 = wp_dma(0, 2)
                nc.sync.dma_start(bqT_t[:], bqT_d)
                x_half_dma(xhi_t, xhi_d, 1, 0, nc.gpsimd)
                wl0 = wvl_dma(0)
                x_half_dma(xhi_t, xhi_d, 2, 0, nc.scalar)
                x_half_dma(xhi_t, xhi_d, 3, 0)
                for q4 in range(4):
                    x_half_dma(xhi_t, xhi_d, q4, 1,
                               nc.scalar if q4 % 2 == 0 else None)
                w0 = {0: wq0, 1: wk0, 2: wv0, "vl": wl0}
                # head-1 k/q ride ahead of x_lo (their chunks run inside the
                # DMA-bound window); v/v_lo follow x_lo (consumed later)
                wk1 = wp_dma(1, 1)
                wq1 = wp_dma(1, 0)
                for q4 in range(4):
                    x_half_dma(xlo_t, xlo_d, q4, 0,
                               nc.scalar if q4 % 2 == 0 else None)
                for q4 in range(4):
                    x_half_dma(xlo_t, xlo_d, q4, 1,
                               nc.scalar if q4 % 2 == 0 else None)
                w1 = {0: wq1, 1: wk1, 2: wp_dma(1, 2),
                      "vl": wvl_dma(1)}
                # preload the ACT Exp table off the critical path
                dummy = small.tile([P, 1], F32, tag="racc", name="dummy")
                nc.scalar.activation(dummy[:], bqT_t[:, 0:1], AF.Exp)

                # ---- projection chunk emitters (head a) ----
                def emit_term(wd, ps, proj, c, term, start, stop):
                    """8 DR matmuls of one (proj, chunk, hi/lo term)."""
                    wt_full = wd["vl"] if term == 1 else wd[proj]
                    xt_full = xlo_t if term == 2 else xhi_t
                    for g in range(8):
                        nc.tensor.matmul(
                            ps[:], wt_full[:, 2 * g:2 * g + 2, :],
                            xt_full[:, 2 * g:2 * g + 2, ts(c, 512)],
                            start=(start and g == 0),
                            stop=(stop and g == 7),
                            perf_mode=DR,
                        )

                def emit_qk_chunk(wd, hl, proj, c):
                    """q or k: single W_hi term; evicted by caller."""
                    ps = ps_a.tile([P, 512], F32, tag="a", name="ps")
                    emit_term(wd, ps, proj, c, 0, True, True)
                    return ps

                def open_v_chunk(wd, c):
                    ps = ps_a.tile([P, 512], F32, tag="a", name="psv")
                    emit_term(wd, ps, 2, c, 0, True, False)
                    return ps

                def evict_chunk(hl, proj, c, ps):
                    if proj == 0:
                        nc.vector.tensor_scalar(
                            q8[:, hl, 0, ts(c, 512)], ps[:],
                            bqT_t[:, hl:hl + 1], QSC8,
                            op0=OP.add, op1=OP.mult,
                        )
                        qtmp = qtp.tile([P, 512], F32, tag="qtmp", name="qtmp")
                        nc.vector.tensor_scalar(
                            qtmp[:], ps[:],
                            bqT_t[:, hl:hl + 1], QSC8,
                            op0=OP.add, op1=OP.mult,
                        )
                        nc.vector.tensor_tensor(
                            q8[:, hl, 1, ts(c, 512)], qtmp[:],
                            q8[:, hl, 0, ts(c, 512)], op=OP.subtract,
                        )
                    elif proj == 1:
                        nc.scalar.activation(
                            k8[:, hl, ts(c, 512)], ps[:],
                            AF.Copy, scale=KSC8,
                        )
                    else:
                        nc.vector.tensor_scalar(
                            vT[:, hl, ts(c, 512)], ps[:],
                            float(ASC), 0.0, op0=OP.mult, op1=OP.add,
                        )
                        nc.sync.dma_start_transpose(
                            v_h[:, hl, 4 * c:4 * c + 4, :],
                            vT[:, hl, ts(c, 512)],
                        )

                # ---- attention step makers (head b) ----
                def make_B(hl, r8=True):
                    ET = etp.tile([P, NT, T], BF16, tag="ET", name="ET")
                    # fp8 copy of the cols>=512 half of ET, for the DR
                    # row-sums of r1 (long rows only: fp8 noise ~eps/sqrt(n))
                    ET8 = (etp.tile([P, NT, 512], FP8, tag="ET8", name="ET8")
                           if r8 else None)
                    rb = rbp.tile([P, T], F32, tag="rb", name="rb")
                    y0 = ps_ar.tile([P, 512], F32, tag="y", name="y0")
                    r0 = ps_ar.tile([P, 512], F32, tag="r", name="r0")
                    y1 = ps_ar.tile([P, 512], F32, tag="y", name="y1")
                    r1 = ps_ar.tile([P, 512], F32, tag="r", name="r1")

                    def CP(j):
                        # only the region where tile j's DR pair-partner is
                        # also post-diagonal; never touches the Pool affine's
                        # output, so ACT never bubbles
                        lo = 512 if j < 4 else 768
                        if j >= 6:
                            return
                        nc.scalar.activation(ET8[:, j, lo - 512:512],
                                             ET[:, j, lo:T], AF.Copy,
                                             scale=float(RC))

                    # cp placement tuned so exps 4-7 aren't pushed past the
                    # single-scores-buffer deadline and each cp lands just
                    # before its r1 consumer
                    CPS_AT_ST = {1: 0, 2: 1, 3: 2, 4: 3, 6: 4, 7: 5}

                    def ST(j):
                        if r8 and j in CPS_AT_ST:
                            CP(CPS_AT_ST[j])
                        # stride-0 repeat of the single k copy: the DR pair
                        # contracts [k; k] against [q_hi; q_lo]
                        kblk = k8[:, hl, ts(j, P)].unsqueeze(1).broadcast_to(
                            [P, 2, P])
                        # two independent 1-bank score tiles with separate
                        # exps: halves the WAR-serialization granularity
                        # between consecutive score tiles
                        if j < 4:
                            sA = ps_s.tile([P, 512], F32, tag="sA", name="sA")
                            nc.tensor.matmul(sA[:, j * P:512], kblk,
                                             q8[:, hl, 0:2, j * P:512],
                                             start=True, stop=True,
                                             perf_mode=DR)
                            nc.scalar.activation(ET[:, j, j * P:512],
                                                 sA[:, j * P:512], AF.Exp,
                                                 scale=float(SSC))
                        sB = ps_s.tile([P, 512], F32, tag="sB", name="sB")
                        lo = max(j * P, 512)
                        nc.tensor.matmul(sB[:, lo - 512:512], kblk,
                                         q8[:, hl, 0:2, lo:T],
                                         start=True, stop=True,
                                         perf_mode=DR)
                        nc.scalar.activation(ET[:, j, lo:T],
                                             sB[:, lo - 512:512], AF.Exp,
                                             scale=float(SSC))
                        nc.gpsimd.affine_select(
                            out=ET[:, j, j * P:(j + 1) * P],
                            in_=ET[:, j, j * P:(j + 1) * P],
                            compare_op=mybir.AluOpType.is_ge, fill=0.0,
                            base=0, pattern=[[1, P]], channel_multiplier=-1,
                        )

                    def AVR(jq):
                        if jq <= 3:
                            lo = jq * P
                            st, sp = jq == 0, jq == 3
                            nc.tensor.matmul(y0[:, lo:512], v_h[:, hl, jq, :],
                                             ET[:, jq, lo:512], start=st, stop=sp,
                                             skip_group_check=True)
                            nc.tensor.matmul(r0[:, lo:512], ones16[:],
                                             ET[:, jq, lo:512], start=st, stop=sp,
                                             skip_group_check=True)
                        lo = max(jq * P, 512)
                        st, sp = jq == 0, jq == NT - 1
                        nc.tensor.matmul(y1[:, lo - 512:512], v_h[:, hl, jq, :],
                                         ET[:, jq, lo:T], start=st, stop=sp,
                                         skip_group_check=True)
                        if not r8:
                            nc.tensor.matmul(r1[:, lo - 512:512], ones16[:],
                                             ET[:, jq, lo:T], start=st, stop=sp,
                                             skip_group_check=True)
                            return
                        # r1 via fp8 DR on adjacent ET8 tile-pairs where both
                        # tiles are post-diagonal; diagonal tiles and tile 6's
                        # unpaired region ride bf16. Emitted at the latest AVR
                        # step whose CPs have landed.
                        if jq == 1:
                            nc.tensor.matmul(
                                r1[:, 0:512],
                                ones8[:].unsqueeze(1).broadcast_to([P, 2, P]),
                                ET8[:, 0:2, 0:512],
                                start=True, stop=False,
                                perf_mode=DR, skip_group_check=True)
                        elif jq == 3:
                            nc.tensor.matmul(
                                r1[:, 0:512],
                                ones8[:].unsqueeze(1).broadcast_to([P, 2, P]),
                                ET8[:, 2:4, 0:512],
                                start=False, stop=False,
                                perf_mode=DR, skip_group_check=True)
                        elif jq == 4:
                            # tile 4's diagonal plus its [640:768] region
                            # (pair (4,5) only covers cols >= 768)
                            nc.tensor.matmul(r1[:, 0:2 * P], ones16[:],
                                             ET[:, 4, 512:768],
                                             start=False, stop=False,
                                             skip_group_check=True)
                        elif jq == 5:
                            nc.tensor.matmul(r1[:, P:2 * P], ones16[:],
                                             ET[:, 5, 640:768],
                                             start=False, stop=False,
                                             skip_group_check=True)
                        elif jq == 6:
                            nc.tensor.matmul(
                                r1[:, 256:512],
                                ones8[:].unsqueeze(1).broadcast_to([P, 2, P]),
                                ET8[:, 4:6, 256:512],
                                start=False, stop=False,
                                perf_mode=DR, skip_group_check=True)
                            nc.tensor.matmul(r1[:, 2 * P:4 * P], ones16[:],
                                             ET[:, 6, 768:T],
                                             start=False, stop=False,
                                             skip_group_check=True)
                        elif jq == 7:
                            nc.tensor.matmul(r1[:, 3 * P:512], ones16[:],
                                             ET[:, 7, 896:T],
                                             start=False, stop=True,
                                             skip_group_check=True)

                    def norm3(dst_hi, dst_lo, y_ps, rb_ap, tag):
                        nc.vector.tensor_tensor(dst_hi, y_ps, rb_ap,
                                                op=OP.mult)
                        ytmp = ytp.tile([P, 512], F32, tag="yt",
                                        name="ytmp" + tag)
                        nc.vector.tensor_tensor(ytmp[:], y_ps, rb_ap,
                                                op=OP.mult)
                        nc.vector.tensor_tensor(dst_lo, ytmp[:], dst_hi,
                                                op=OP.subtract)

                    def tail0():
                        nc.vector.reciprocal(rb[:, 0:512], r0[:])
                        norm3(yhi[:, hl, 0:512], ylo[:, hl, 0:512],
                              y0[:], rb[:, 0:512], "0")

                    def tail1():
                        nc.vector.reciprocal(rb[:, 512:T], r1[:])
                        norm3(yhi[:, hl, 512:T], ylo[:, hl, 512:T],
                              y1[:], rb[:, 512:T], "1")

                    return ST, AVR, tail0, tail1

                # pre-issue the first three wot blocks early (they gate the
                # B(7)-trail weave and C start)
                wot_pre = [wot_dma(ob) for ob in range(2)]

                # ---- out-projection chunk helpers (shared with phase C) ----
                def emit_C_mms(wt_hi, wt_lo, c, o_ps):
                    for term in range(3):  # hh, lh, hl
                        wt = wt_lo if term == 1 else wt_hi
                        yt = ylo if term == 2 else yhi
                        for g in range(4):
                            nc.tensor.matmul(
                                o_ps[:],
                                wt[:, 2 * g:2 * g + 2, :],
                                yt[:, 2 * g:2 * g + 2, ts(c, 512)],
                                start=(term == 0 and g == 0),
                                stop=(term == 2 and g == 3),
                                perf_mode=DR,
                            )

                # ---- prologue: heads 0+1 under the DMA-bound window ----
                # Slot 0 sweeps quad-major with six open PSUM chunks (the
                # y/r banks are idle before attention starts); head 1's
                # x_hi-only chunks slip in before head 0's x_lo-fed terms so
                # PE never waits on the tail of the input stream.
                kc = [ps_a.tile([P, 512], F32, tag="a", name="kc")
                      for _ in range(2)]
                qc = [ps_ar.tile([P, 512], F32, tag="y", name="qc")
                      for _ in range(2)]
                vc = [ps_ar.tile([P, 512], F32, tag="r", name="vc")
                      for _ in range(2)]

                def mm0(ps, wt, g, c, xt, start, stop):
                    nc.tensor.matmul(
                        ps[:], wt[:, 2 * g:2 * g + 2, :],
                        xt[:, 2 * g:2 * g + 2, ts(c, 512)],
                        start=start, stop=stop, perf_mode=DR)

                for c in range(2):
                    for g in range(8):
                        mm0(kc[c], w0[1], g, c, xhi_t, g == 0, g == 7)
                        mm0(qc[c], w0[0], g, c, xhi_t, g == 0, g == 7)
                        mm0(vc[c], w0[2], g, c, xhi_t, g == 0, False)
                    evict_chunk(0, 1, c, kc[c])
                    evict_chunk(0, 0, c, qc[c])
                for c in range(2):
                    for g in range(8):
                        mm0(vc[c], w0["vl"], g, c, xhi_t, False, False)
                # B(0)'s first score tiles start as soon as head 0's q/k are
                # evicted, and head 0's x_lo-fed v terms run chunk-major so
                # each v half evicts + transposes at the earliest moment (the
                # transpose gates B(0)'s P@V steps). Head 1's first chunks
                # fill the x_lo arrival gaps.
                ST, AVR, tail0, tail1 = make_B(0)
                ST(0)
                for g in range(2):
                    mm0(vc[0], w0[2], g, 0, xlo_t, False, False)
                k10 = emit_qk_chunk(w1, 1, 1, 0)
                evict_chunk(1, 1, 0, k10)
                for g in range(2, 4):
                    mm0(vc[0], w0[2], g, 0, xlo_t, False, False)
                ST(1)
                q10 = emit_qk_chunk(w1, 1, 0, 0)
                evict_chunk(1, 0, 0, q10)
                for g in range(4, 8):
                    mm0(vc[0], w0[2], g, 0, xlo_t, False, g == 7)
                evict_chunk(0, 2, 0, vc[0])
                ST(2)
                for g in range(8):
                    mm0(vc[1], w0[2], g, 1, xlo_t, False, g == 7)
                evict_chunk(0, 2, 1, vc[1])
                w2 = w_head_dma(2)
                # head 1 remainder woven with head 0's attention
                k11 = emit_qk_chunk(w1, 1, 1, 1)
                ST(3)
                evict_chunk(1, 1, 1, k11)
                psv = open_v_chunk(w1, 0)
                AVR(0)
                emit_term(w1, psv, 2, 0, 1, False, False)
                AVR(1)
                ST(4)
                emit_term(w1, psv, 2, 0, 2, False, True)
                AVR(2)
                evict_chunk(1, 2, 0, psv)
                q11 = emit_qk_chunk(w1, 1, 0, 1)
                ST(5)
                evict_chunk(1, 0, 1, q11)
                AVR(3)
                tail0()
                psv = open_v_chunk(w1, 1)
                ST(6)
                AVR(4)
                emit_term(w1, psv, 2, 1, 1, False, False)
                ST(7)
                AVR(5)
                emit_term(w1, psv, 2, 1, 2, False, True)
                evict_chunk(1, 2, 1, psv)
                AVR(6)
                AVR(7)
                tail1()
                Bst = make_B(1)
                wd = w2

                # ---- the steady-state slot loop (heads 2..7) ----
                for a in range(2, HL):
                    if a + 1 < HL:
                        wn = w_head_dma(a + 1)  # prefetch next head (SP)
                    # weave head a's projections with head a-1's
                    # attention. k-evictions go on ACT ahead of the exps
                    # (the v chunk reuses k's PSUM bank); STs spaced by
                    # A-matmuls so the single scores buffer never gates.
                    ST, AVR, tail0, tail1 = Bst
                    ps = emit_qk_chunk(wd, a, 1, 0)  # k c0
                    evict_chunk(a, 1, 0, ps)
                    ps = emit_qk_chunk(wd, a, 0, 0)  # q c0
                    ST(0)
                    evict_chunk(a, 0, 0, ps)
                    psv = open_v_chunk(wd, 0)        # v c0 hh
                    ST(1)
                    AVR(0)
                    emit_term(wd, psv, 2, 0, 1, False, False)
                    ST(2)
                    AVR(1)
                    emit_term(wd, psv, 2, 0, 2, False, True)
                    ST(3)
                    evict_chunk(a, 2, 0, psv)
                    AVR(2)
                    ps = emit_qk_chunk(wd, a, 1, 1)  # k c1
                    ST(4)
                    evict_chunk(a, 1, 1, ps)
                    AVR(3)
                    tail0()
                    ps = emit_qk_chunk(wd, a, 0, 1)  # q c1
                    ST(5)
                    evict_chunk(a, 0, 1, ps)
                    AVR(4)
                    psv = open_v_chunk(wd, 1)        # v c1 hh
                    ST(6)
                    AVR(5)
                    emit_term(wd, psv, 2, 1, 1, False, False)
                    ST(7)
                    AVR(6)
                    emit_term(wd, psv, 2, 1, 2, False, True)
                    evict_chunk(a, 2, 1, psv)
                    AVR(7)
                    tail1()
                    # last head's attention runs un-fused (exp-chain bound):
                    # keep its row-sums on the cheap-latency bf16 path there
                    Bst = make_B(a, r8=(a < HL - 1))
                    if a + 1 < HL:
                        wd = wn

                # trailing attention for the last head; once its first-chunk
                # normalize lands, weave in the first two out-proj chunks to
                # hide the exp chain
                ST, AVR, tail0, tail1 = Bst
                ST(0)
                ST(1)
                AVR(0)
                ST(2)
                AVR(1)
                ST(3)
                AVR(2)
                ST(4)
                AVR(3)
                tail0()
                ST(5)
                AVR(4)
                o00 = ps_a.tile([P, 512], F32, tag="a", name="o00")
                emit_C_mms(wot_pre[0][0], wot_pre[0][1], 0, o00)
                ST(6)
                AVR(5)
                o10 = ps_a.tile([P, 512], F32, tag="a", name="o10")
                emit_C_mms(wot_pre[1][0], wot_pre[1][1], 0, o10)
                ST(7)
                AVR(6)
                AVR(7)
                tail1()
                for ob, o_ps in ((0, o00), (1, o10)):
                    out_sb = ytp.tile([P, 512], BF16, tag="osb0", name="osb0")
                    nc.scalar.activation(out_sb[:], o_ps[:], AF.Copy,
                                         scale=float(CSC))
                    nc.sync.dma_start(outT_d[ob, :, 0:512], out_sb[:])

            # -------- Phase C: partial out-projection (fp8 DR) --------
            with (
                tc.tile_pool(name="osb", bufs=4) as osb,
                tc.tile_pool(name="ps_o", bufs=4, space="PSUM") as ps_o,
            ):
                # (0,0)/(1,0) were woven into the B(7) trail; (0,1)/(1,1)
                # run last since their wot is prefetched (no DMA dependency
                # near the tail) and (0,1) must wait for the trail's second
                # normalize anyway
                # c=0 chunks lead: their y inputs are final at the trail's
                # first normalize, while c=1 needs the second normalize
                # which is still draining on DVE when C starts
                chunk_order = [(2, 0), (3, 0), (4, 0), (2, 1), (3, 1), (4, 1)]
                chunk_order += [(ob, c) for ob in range(5, ND) for c in range(2)]
                chunk_order += [(0, 1), (1, 1)]
                wot_t = dict(enumerate(wot_pre))
                for ob, c in chunk_order:
                    if ob not in wot_t:
                        wot_t[ob] = wot_dma(ob)
                    wt_hi, wt_lo = wot_t[ob]
                    if True:
                        o_ps = ps_o.tile([P, 512], F32, tag="o")
                        for term in range(3):  # hh, lh, hl
                            wt = wt_lo if term == 1 else wt_hi
                            yt = ylo if term == 2 else yhi
                            for g in range(4):
                                nc.tensor.matmul(
                                    o_ps[:],
                                    wt[:, 2 * g:2 * g + 2, :],
                                    yt[:, 2 * g:2 * g + 2, ts(c, 512)],
                                    start=(term == 0 and g == 0),
                                    stop=(term == 2 and g == 3),
                                    perf_mode=DR,
                                )
                        last = (ob, c) == chunk_order[-1]
                        # the final chunk evicts in halves so its DMA-latency
                        # chain starts one copy earlier
                        for h in range(2 if last else 1):
                            hs = slice(h * 256, 256 + h * 256) if last \
                                else slice(0, 512)
                            out_sb = osb.tile([P, 512], BF16, tag="osb")
                            nc.scalar.activation(out_sb[:, hs], o_ps[:, hs],
                                                 AF.Copy, scale=float(CSC))
                            nc.sync.dma_start(
                                outT_d[ob, :, 512 * c + hs.start:
                                       512 * c + hs.stop],
                                out_sb[:, hs])

    nc.compile()
    return nc


def _get_compiled():
    if "nc" not in _cache:
        _cache["nc"] = _build()
    return _cache["nc"]


def _hilo(a, e):
    import ml_dtypes
    F8 = ml_dtypes.float8_e4m3
    s = np.asarray(a, np.float32) * np.float32(2.0 ** e)
    hi = s.astype(F8)
    lo = (s - hi.astype(np.float32)).astype(F8)
    return hi, lo


def _host_prep(x, Wq, bq, Wk, Wv, Wo):
    """Build per-core input maps."""
    in_maps = []
    # xT per batch: [D, T] -> [ND, P, T] fp8 hi/lo at scale 2^EX
    xTs = []
    for b in range(B):
        xh, xl = _hilo(np.ascontiguousarray(x[b].T).reshape(ND, P, T), EX)
        xTs.append((xh, xl))
    halves = []
    for half in range(2):
        hs = slice(half * HL, (half + 1) * HL)
        # WqT/WkT/WvT per head: [D, E] -> [ND, P, E]; pack so each 4-d-tile
        # group is one contiguous [P, 4*3P] DMA: [HL, 4, P, 4*3*P].
        # The lo stream only carries v (q/k are W_hi-only): [HL, 4, P, 4*P].
        w3s = np.empty((HL, ND, P, 3 * P), dtype=np.float32)
        for hl, h in enumerate(range(half * HL, (half + 1) * HL)):
            w3s[hl, :, :, 0:P] = Wq[h].T.reshape(ND, P, P)
            w3s[hl, :, :, P:2 * P] = Wk[h].T.reshape(ND, P, P)
            w3s[hl, :, :, 2 * P:3 * P] = Wv[h].T.reshape(ND, P, P)
        whi_s, wlo_s = _hilo(w3s, EW)
        # proj-major flat packs: [HL, 3, P, ND*P] hi, [HL, P, ND*P] v-lo
        whi = np.ascontiguousarray(
            np.asarray(whi_s).reshape(HL, ND, P, 3, P)
            .transpose(0, 3, 2, 1, 4)
        ).reshape(HL, 3, P, ND * P)
        wlo = np.ascontiguousarray(
            np.asarray(wlo_s)[:, :, :, 2 * P:3 * P].transpose(0, 2, 1, 3)
        ).reshape(HL, P, ND * P)
        bqT = np.ascontiguousarray(bq[hs].T) * np.float32(2.0 ** (EX + EW))
        # WoT span blocks: WoT = Wo.T [i, o]; rows i in this half's span
        WoT_span = Wo.T[half * 1024:(half + 1) * 1024]  # [1024, D]
        # pack to [ND(o-block), P, HL*P] so each o-block is one contiguous DMA
        wot = np.ascontiguousarray(
            WoT_span.reshape(HL, P, ND, P).transpose(2, 1, 0, 3)
        ).reshape(ND, P, HL * P)
        wothi, wotlo = _hilo(wot, EWO)
        wotp = np.ascontiguousarray(
            np.stack([np.asarray(wothi), np.asarray(wotlo)], axis=1))
        halves.append({"whi": whi, "wlo": wlo, "bqT": bqT, "wot": wotp})
    for c in range(8):
        b, half = c // 2, c % 2
        hv = halves[half]
        in_maps.append({"xhi": xTs[b][0], "xlo": xTs[b][1], "whi": hv["whi"],
                        "wlo": hv["wlo"], "bqT": hv["bqT"],
                        "wot": hv["wot"]})
    return in_maps


def _numpy_fallback(x, attention_mask, Wq, bq, Wk, bk, Wv, bv, Wo, bo):
    out = np.empty((B, T, D), dtype=np.float32)
    neg = np.float32(np.finfo(np.float32).min)
    for b in range(B):
        xb = x[b]
        q = np.einsum("td,hed->hte", xb, Wq) + bq[:, None, :]
        k = np.einsum("td,hed->hte", xb, Wk) + bk[:, None, :]
        v = np.einsum("td,hed->hte", xb, Wv) + bv[:, None, :]
        s = np.einsum("hqe,hke->hqk", q, k).astype(np.float32) * np.float32(SCALE)
        causal = np.arange(T)[None, :] > np.arange(T)[:, None]
        s = np.where(causal[None], neg, s)
        keep = attention_mask[b].astype(bool)
        s = np.where(keep[None, None, :], s, neg)
        s = s - s.max(-1, keepdims=True)
        p = np.exp(s)
        p = p / p.sum(-1, keepdims=True)
        y = np.einsum("hqk,hke->hqe", p, v)
        y = np.transpose(y, (1, 0, 2)).reshape(T, D)
        out[b] = y @ Wo.T + bo
    return out


def kernel(x, attention_mask, Wq, bq, Wk, bk, Wv, bv, Wo, bo):
    x = np.asarray(x, dtype=np.float32)
    attention_mask = np.asarray(attention_mask)
    Wq, bq = np.asarray(Wq, np.float32), np.asarray(bq, np.float32)
    Wk, bk = np.asarray(Wk, np.float32), np.asarray(bk, np.float32)
    Wv, bv = np.asarray(Wv, np.float32), np.asarray(bv, np.float32)
    Wo, bo = np.asarray(Wo, np.float32), np.asarray(bo, np.float32)

    if not np.all(attention_mask == 1):
        return _numpy_fallback(x, attention_mask, Wq, bq, Wk, bk, Wv, bv, Wo, bo)

    from concourse.bass_utils import run_bass_kernel_spmd

    nc = _get_compiled()
    in_maps = _host_prep(x, Wq, bq, Wk, Wv, Wo)
    res = run_bass_kernel_spmd(nc, in_maps, core_ids=list(range(8)))

    # bv folds through softmax (rows sum to 1); bk is softmax-invariant
    bo_total = (bo + Wo @ bv.reshape(D)).astype(np.float32)

    out = np.zeros((B, T, D), dtype=np.float32)
    for c in range(8):
        partial = np.asarray(res.results[c]["outT"]).astype(np.float32)
        out[c // 2] += partial.reshape(D, T).T
    out += bo_total
    return out
